# revision 1
# baseline (speedup 1.0000x reference)
"""LID detector kernel for Trainium2 (8 NeuronCores, data-parallel over batch).

Per core (batch shard of 32):
  - mean-pool each feature map over space -> q [C, 32] (transposed layout)
  - -d2 = 2*q.r - ||r||^2 - ||q||^2 via PE matmuls into PSUM, evicted into a
    stacked [128, 2000] buffer (partition quadrant = layer)
  - top-24 smallest d2 via 3 rounds of DVE max8 + match_replace
  - LID = -2k / (sum_{i=1..20} ln d2_i - 20 ln d2_20)  (no sqrt needed)
  - logit = w . lid + b -> sigmoid -> out [32]
"""

import sys

for _p in ("/opt/trn_rl_repo", "/root/.axon_site/_ro/trn_rl_repo"):
    if _p not in sys.path:
        sys.path.append(_p)

import ml_dtypes
import numpy as np

import concourse.mybir as mybir
from concourse import bass, bacc
from concourse.tile import TileContext
from concourse.bass_utils import run_bass_kernel_spmd

F32 = mybir.dt.float32
BF16 = mybir.dt.bfloat16
N_CORES = 8
B = 32  # batch shard per core
R = 2000
K = 20
LAYERS = [(64, 3136), (128, 784), (256, 196), (512, 49)]  # (C, H*W)
NEG_BIG = -3.0e38

# column j of qT holds sample SIGMA[j] of the local shard
SIGMA = np.array([2 * j for j in range(16)] + [2 * j + 1 for j in range(16)])


def build_nc():
    nc = bacc.Bacc("TRN2", target_bir_lowering=False, debug=False,
                   num_devices=N_CORES)

    feats = [nc.dram_tensor(f"feat{l}", [B, C, HW], BF16, kind="ExternalInput")
             for l, (C, HW) in enumerate(LAYERS)]
    refTs = [nc.dram_tensor(f"refT{l}", [C, R], F32, kind="ExternalInput")
             for l, (C, _) in enumerate(LAYERS)]
    regw = nc.dram_tensor("regw", [1, 4], F32, kind="ExternalInput")
    regb = nc.dram_tensor("regb", [1, 1], F32, kind="ExternalInput")
    out = nc.dram_tensor("out", [B, 1], F32, kind="ExternalOutput")
    import os
    _dbg = os.environ.get("DEBUG_LID") == "1"
    if _dbg:
        dbg_lid = nc.dram_tensor("dbg_lid", [128, 1], F32, kind="ExternalOutput")
        dbg_vals = nc.dram_tensor("dbg_vals", [128, 24], F32, kind="ExternalOutput")
        dbg_q = nc.dram_tensor("dbg_q", [64, B], F32, kind="ExternalOutput")
        dbg_tk = nc.dram_tensor("dbg_tk", [128, R], F32, kind="ExternalOutput")
        dbg_rn2a = nc.dram_tensor("dbg_rn2a", [65, R], F32, kind="ExternalOutput")
        dbg_rn2b = nc.dram_tensor("dbg_rn2b", [1, R], F32, kind="ExternalOutput")

    with TileContext(nc) as tc:
        with (
            tc.tile_pool(name="persist", bufs=1) as pp,
            tc.tile_pool(name="ft", bufs=6) as fp,
            tc.tile_pool(name="sq", bufs=2) as sqp,
        ):
            # ---- persistent tiles
            rt = {}   # (l, i) -> refT chunk tile [Cc, R]
            for l, (C, _) in enumerate(LAYERS):
                for i in range(0, C, 128):
                    Cc = min(128, C - i)
                    rt[(l, i)] = pp.tile([Cc, R], F32, tag=f"rt{l}_{i}",
                                         name=f"rt{l}_{i}")
            act_scratch = pp.tile([128, 3136], BF16, tag="act_scratch",
                                  name="act_scratch")
            rn2a = pp.tile([65, R], F32, tag="rn2a", name="rn2a")
            rn2b = pp.tile([1, R], F32, tag="rn2b", name="rn2b")
            rn2base = [(rn2a, 0), (rn2a, 32), (rn2a, 64), (rn2b, 0)]
            rn2row = [t[b:b + 1, :] for (t, b) in rn2base]
            qT = {}
            for l, (C, _) in enumerate(LAYERS):
                for i in range(0, C, 128):
                    Cc = min(128, C - i)
                    qT[(l, i)] = pp.tile([Cc, B], F32, tag=f"qT{l}_{i}", name=f"qT{l}_{i}")
            qn2neg = [pp.tile([B, 1], F32, tag=f"qn2_{l}", name=f"qn2_{l}") for l in range(4)]
            topkbuf = pp.tile([128, R], F32, tag="topkbuf", name="topkbuf")
            vals = pp.tile([128, 24], F32, tag="vals", name="vals")
            ones_col = pp.tile([128, 1], F32, tag="ones_col", name="ones_col")
            ones_row = pp.tile([1, B], F32, tag="ones_row", name="ones_row")
            negones_all = pp.tile([65, B], F32, tag="negones_all", name="negones_all")
            wb_sb = pp.tile([1, 5], F32, tag="wb_sb", name="wb_sb")
            tmp0 = pp.tile([128, 16], F32, tag="tmp0", name="tmp0")

            nc.vector.memset(ones_col[:], 1.0)
            nc.vector.memset(ones_row[:], 1.0)
            nc.vector.memset(negones_all[:], -1.0)
            nc.sync.dma_start(out=wb_sb[0:1, 0:4], in_=regw[:])
            nc.sync.dma_start(out=wb_sb[0:1, 4:5], in_=regb[:])

            # ---- ref squared norms: rn2[l] [1, R] = sum_c refT^2
            with tc.tile_pool(name="psumA", bufs=1,
                              space=bass.MemorySpace.PSUM) as pA:
                for l, (C, _) in enumerate(LAYERS):
                    ps = pA.tile([1, R], F32, tag="rn2ps", name="rn2ps")
                    chunks = list(range(0, C, 128))
                    for ci, i in enumerate(chunks):
                        Cc = min(128, C - i)
                        sq = sqp.tile([128, R], F32, tag="sq", name="sq")
                        nc.scalar.square(sq[0:Cc, :], rt[(l, i)][:])
                        for c0 in range(0, R, 512):
                            n = min(512, R - c0)
                            nc.tensor.matmul(
                                ps[0:1, c0:c0 + n],
                                ones_col[0:Cc, 0:1],
                                sq[0:Cc, c0:c0 + n],
                                start=(ci == 0), stop=(ci == len(chunks) - 1),
                            )
                    nc.scalar.copy(rn2row[l], ps[:])

            # ---- pooling: fill qT columns (sample order SIGMA)
            # layer 0: C=64, 2 samples per 128 partitions
            C, HW = LAYERS[0]
            for t in range(8):
                tile = fp.tile([128, 2, HW], BF16, tag="ft", name="ft")
                src = bass.AP(feats[0], 4 * t * C * HW,
                              [[HW, 128], [2 * C * HW, 2], [1, HW]])
                nc.sync.dma_start(out=tile[:], in_=src)
                if t < 4:
                    nc.vector.tensor_reduce(
                        tmp0[:, 2 * t:2 * t + 2], tile[:],
                        axis=mybir.AxisListType.X, op=mybir.AluOpType.add)
                else:
                    for g in range(2):
                        nc.scalar.activation(
                            act_scratch[:, 0:HW], tile[:, g, :],
                            mybir.ActivationFunctionType.Copy,
                            accum_out=tmp0[:, 2 * t + g:2 * t + g + 1])
            nc.vector.tensor_copy(qT[(0, 0)][:, 0:16], tmp0[0:64, :])
            nc.vector.tensor_copy(qT[(0, 0)][:, 16:32], tmp0[64:128, :])

            # layers 1..3: per chunk, DMA samples with stride 2 (even then odd)
            for l in (1, 2, 3):
                C, HW = LAYERS[l]
                n_chunks = C // 128
                g = B // (2 * 4 // 1)  # placeholder, set below
                # samples per DMA: L1:4 (8 DMAs), L2:8 (4 DMAs), L3:16 (2 DMAs)
                spd = {1: 4, 2: 8, 3: 16}[l]
                ndma = B // spd
                for i in range(n_chunks):
                    for t in range(ndma):
                        # cols spd*t .. spd*t+spd-1 -> samples SIGMA[col]
                        # = base + 2*j, base = 2*spd*t if even half else ...
                        col0 = spd * t
                        s_base = int(SIGMA[col0])
                        tile = fp.tile([128, spd, HW], BF16, tag="ft", name="ft")
                        src = bass.AP(
                            feats[l],
                            s_base * C * HW + 128 * i * HW,
                            [[HW, 128], [2 * C * HW, spd], [1, HW]])
                        nc.sync.dma_start(out=tile[:], in_=src)
                        if l == 1:
                            for g in range(spd):
                                nc.scalar.activation(
                                    act_scratch[:, 0:HW], tile[:, g, :],
                                    mybir.ActivationFunctionType.Copy,
                                    accum_out=qT[(l, 128 * i)][:, col0 + g:col0 + g + 1])
                        else:
                            nc.vector.tensor_reduce(
                                qT[(l, 128 * i)][:, col0:col0 + spd], tile[:],
                                axis=mybir.AxisListType.X, op=mybir.AluOpType.add)

            for l, (C, _) in enumerate(LAYERS):
                for i in range(0, C, 128):
                    Cc = min(128, C - i)
                    nc.sync.dma_start(out=rt[(l, i)][:],
                                      in_=refTs[l][i:i + Cc, :])

            # ---- scale qT by 2/HW (so lhsT holds 2*q), qn2neg
            with tc.tile_pool(name="psumB", bufs=1,
                              space=bass.MemorySpace.PSUM) as pB:
                for l, (C, HW) in enumerate(LAYERS):
                    chunks = list(range(0, C, 128))
                    qps = pB.tile([B, 1], F32, tag="qn2ps", name="qn2ps")
                    for ci, i in enumerate(chunks):
                        Cc = min(128, C - i)
                        nc.scalar.mul(qT[(l, i)][:], qT[(l, i)][:], 2.0 / HW)
                        qsq = sqp.tile([128, B], F32, tag="qsq", name="qsq")
                        # (2q * 0.5)^2 = q^2
                        nc.scalar.activation(
                            qsq[0:Cc, :], qT[(l, i)][:],
                            mybir.ActivationFunctionType.Square, scale=0.5)
                        nc.tensor.matmul(
                            qps[:], qsq[0:Cc, :], ones_col[0:Cc, 0:1],
                            start=(ci == 0), stop=(ci == len(chunks) - 1))
                    nc.scalar.mul(qn2neg[l][:], qps[:], -1.0)

                # ---- distances: psum = 2q.r - rn2 ; evict + qn2neg -> -d2
                for l, (C, _) in enumerate(LAYERS):
                    chunks = list(range(0, C, 128))
                    for c0 in range(0, R, 512):
                        n = min(512, R - c0)
                        dps = pB.tile([B, 512], F32, tag="d2ps", name="d2ps")
                        for ci, i in enumerate(chunks):
                            Cc = min(128, C - i)
                            nc.tensor.matmul(
                                dps[:, 0:n], qT[(l, i)][:],
                                rt[(l, i)][:, c0:c0 + n],
                                start=(ci == 0), stop=False)
                        rn2t, rn2b_ = rn2base[l]
                        nc.tensor.matmul(
                            dps[:, 0:n], negones_all[rn2b_:rn2b_ + 1, :],
                            rn2t[rn2b_:rn2b_ + 1, c0:c0 + n],
                            start=False, stop=True)
                        nc.vector.tensor_scalar(
                            topkbuf[32 * l:32 * l + 32, c0:c0 + n],
                            dps[:, 0:n], qn2neg[l][:], None,
                            op0=mybir.AluOpType.add)

                if _dbg:
                    nc.sync.dma_start(out=dbg_tk[:], in_=topkbuf[:])
                    nc.sync.dma_start(out=dbg_rn2a[:], in_=rn2a[:])
                    nc.sync.dma_start(out=dbg_rn2b[:], in_=rn2b[:])
                # ---- top-24 (ascending d2 == descending -d2)
                nc.vector.max(vals[:, 0:8], topkbuf[:])
                nc.vector.match_replace(topkbuf[:], vals[:, 0:8], topkbuf[:],
                                        NEG_BIG)
                nc.vector.max(vals[:, 8:16], topkbuf[:])
                nc.vector.match_replace(topkbuf[:], vals[:, 8:16], topkbuf[:],
                                        NEG_BIG)
                nc.vector.max(vals[:, 16:24], topkbuf[:])

                # ---- LID
                ln2 = pp.tile([128, 24], F32, tag="ln2", name="ln2")
                S = pp.tile([128, 1], F32, tag="S", name="S")
                denom = pp.tile([128, 1], F32, tag="denom", name="denom")
                lid = pp.tile([128, 1], F32, tag="lid", name="lid")
                # clamp: vals <= -1e-30 so that -vals >= 1e-30
                nc.vector.tensor_scalar_min(vals[:], vals[:], -1e-30)
                nc.scalar.activation(ln2[:], vals[:],
                                     mybir.ActivationFunctionType.Ln,
                                     scale=-1.0)
                nc.vector.tensor_reduce(S[:], ln2[:, 1:21],
                                        axis=mybir.AxisListType.X,
                                        op=mybir.AluOpType.add)
                # denom = -20*ln2[20] + S  (= sum ln d2_i - 20 ln d2_20)
                nc.vector.tensor_scalar(denom[:], ln2[:, 20:21], -20.0, S[:],
                                        op0=mybir.AluOpType.mult,
                                        op1=mybir.AluOpType.add)
                nc.vector.reciprocal(lid[:], denom[:])
                nc.vector.tensor_scalar_mul(lid[:], lid[:], -2.0 * K)
                if _dbg:
                    nc.sync.dma_start(out=dbg_lid[:], in_=lid[:])
                    nc.sync.dma_start(out=dbg_vals[:], in_=vals[:])
                    nc.sync.dma_start(out=dbg_q[:], in_=qT[(0, 0)][:])

                # ---- regression + sigmoid
                lid4 = pp.tile([B, 4], F32, tag="lid4", name="lid4")
                for l in range(4):
                    nc.vector.tensor_copy(lid4[:, l:l + 1],
                                          lid[32 * l:32 * l + 32, :])
                wps = pB.tile([B, 5], F32, tag="wps", name="wps")
                nc.tensor.matmul(wps[:], ones_row[:], wb_sb[:],
                                 start=True, stop=True)
                wbc = pp.tile([B, 5], F32, tag="wbc", name="wbc")
                nc.scalar.copy(wbc[:], wps[:])
                prod = pp.tile([B, 4], F32, tag="prod", name="prod")
                nc.vector.tensor_tensor(prod[:], lid4[:], wbc[:, 0:4],
                                        op=mybir.AluOpType.mult)
                ssum = pp.tile([B, 1], F32, tag="ssum", name="ssum")
                nc.vector.tensor_reduce(ssum[:], prod[:],
                                        axis=mybir.AxisListType.X,
                                        op=mybir.AluOpType.add)
                res = pp.tile([B, 1], F32, tag="res", name="res")
                nc.scalar.activation(res[:], ssum[:],
                                     mybir.ActivationFunctionType.Sigmoid,
                                     bias=wbc[:, 4:5])
                nc.sync.dma_start(out=out[:], in_=res[:])

    nc.compile()
    return nc


_NC = None


def _get_nc():
    global _NC
    if _NC is None:
        _NC = build_nc()
    return _NC


def run(trace=False, **inputs):
    nc = _get_nc()
    feats = [np.asarray(inputs[f"feat{l}"], dtype=np.float32) for l in range(4)]
    refTs = [np.ascontiguousarray(np.asarray(inputs[f"ref{l}"],
                                             dtype=np.float32).T)
             for l in range(4)]
    regw = np.asarray(inputs["reg_w"], dtype=np.float32).reshape(1, 4)
    regb = np.asarray(inputs["reg_b"], dtype=np.float32).reshape(1, 1)
    assert int(inputs.get("k", K)) == K

    in_maps = []
    for c in range(N_CORES):
        m = {}
        for l, (C, HW) in enumerate(LAYERS):
            m[f"feat{l}"] = np.ascontiguousarray(
                feats[l][c * B:(c + 1) * B].reshape(B, C, HW)).astype(
                    ml_dtypes.bfloat16)
            m[f"refT{l}"] = refTs[l]
        m["regw"] = regw
        m["regb"] = regb
        in_maps.append(m)

    res = run_bass_kernel_spmd(nc, in_maps, core_ids=list(range(N_CORES)),
                               trace=trace)
    full = np.empty((N_CORES * B,), dtype=np.float32)
    for c in range(N_CORES):
        shard = np.empty((B,), dtype=np.float32)
        shard[SIGMA] = res.results[c]["out"][:, 0]
        full[c * B:(c + 1) * B] = shard
    return full, res


def kernel(**inputs):
    return run(trace=False, **inputs)[0]

